# revision 33
# baseline (speedup 1.0000x reference)
"""Trainium2 Bass kernel for nn_MiM_v2 (Mamba-style selective scan).

Sharding: pure data-parallel over batch B=16 across 8 NeuronCores
(2 batches per core, weights replicated, no collectives).

Per-core pipeline over token chunks (TC tokens at a time):
  fused in_proj+causal-conv: 3 shifted bf16 matmuls with conv-prescaled
    weights + folded bias, x loaded with a 2-token halo (PE only)
  -> silu via tanh: h = ph*(1+tanh(ph)), ph prescaled by 0.5 (ACT+DVE STT;
     tanh shares the exp_and_others table set - no table thrash)
  -> x_proj (PE) -> grouped RMSNorm (PE reduce + ACT ln/exp + DVE)
  -> dt_proj (PE); softplus split into phases (all Exp, then all Ln) so
     the ACT table set flips only twice per chunk
  -> deltaA = exp(A*delta) (ACT, per-partition scale, bf16)
  -> BX = delta*h*B (DVE, broadcast AP vs DRAM-bounced B/C broadcast)
  -> linear recurrence via DVE tensor_tensor_scan, double-buffered
     dA/hs/bx pools for cross-iteration overlap; cross-chunk carries via
     per-n init columns copied from the previous chunk's scan tails
  -> y = sum_n hs*C (DVE mult + gpsimd/DVE tree reduce)  [+ D_skip*h]
  -> out_proj (bf16 PE) -> DMA out
"""

import sys

if "/opt/trn_rl_repo" not in sys.path:
    sys.path.insert(0, "/opt/trn_rl_repo")

import numpy as np
import ml_dtypes

import concourse.bass as bass
import concourse.mybir as mybir
import concourse.tile as tile
from concourse import bacc

# ---------------------------------------------------------------- constants
B, L, DM = 16, 1024, 512
DIN, DT, N, K = 2 * DM, 32, 16, 3
NCORES = 8
BPC = B // NCORES          # batches per core
T = BPC * L                # tokens per core
TC = 512                   # token chunk
NCH = T // TC              # chunks per core
CPB = L // TC              # chunks per batch
NDT = DIN // 128           # d-inner tiles
NKT = DM // 128            # k tiles for in_proj
NH = 2                     # n-groups for scan phase
GP_SCANS = 0               # gpsimd cannot run tensor_tensor_scan (codegen)
NPH = N // NH              # n per half (8)

F32 = mybir.dt.float32
F32R = mybir.dt.float32r
BF16 = mybir.dt.bfloat16
AF = mybir.ActivationFunctionType
ALU = mybir.AluOpType

FOLD_D_HOST = True         # fold D_skip into an extra out_proj matmul


USE_F32R = True
MMDT = None  # set below


MMDT = F32R if USE_F32R else F32


def _r(ap):
    """matmul operands already carry the right dtype; no bitcast needed"""
    return ap


# ---------------------------------------------------------------- host prep
def host_weights(inp):
    """Precompute transposed/reorganized weights (numpy, shared by all cores)."""
    f = lambda x: np.ascontiguousarray(np.asarray(x, np.float32))
    w = {}
    # conv folded into in_proj: h_conv[t,d] = sum_k cw[d,k]*(x[t+k-2]@in_w.T[:,d] + in_b[d]) + conv_b[d]
    # scaled by 0.5 so that silu(x)=x*sigmoid(x)=ph*(1+tanh(ph)) with ph=0.5*x
    cw = np.asarray(inp["conv_w"], np.float32)[:, 0, :]   # (DIN, 3)
    in_wT = np.asarray(inp["in_w"], np.float32).T          # (DM, DIN)
    for k in range(K):
        w[f"wk{k}"] = np.ascontiguousarray(
            (0.5 * in_wT * cw[None, :, k]).astype(ml_dtypes.bfloat16))
    in_b = np.asarray(inp["in_b"], np.float32)
    conv_b = np.asarray(inp["conv_b"], np.float32)
    w["b_eff"] = f(0.5 * (in_b * cw.sum(1) + conv_b))[None, :]       # (1, DIN)
    w["bfix"] = f(np.stack([-0.5 * in_b * (cw[:, 0] + cw[:, 1]),
                            -0.5 * in_b * cw[:, 0]]))               # (2, DIN)
    w["eye2"] = f(np.eye(2, dtype=np.float32))
    w["w_x_T"] = np.ascontiguousarray(
        np.asarray(inp["xproj_w"], np.float32).T.astype(ml_dtypes.bfloat16))
    w["w_dt_T"] = np.ascontiguousarray(
        np.asarray(inp["dt_w"], np.float32).T.astype(ml_dtypes.bfloat16))
    w["w_out_T"] = np.ascontiguousarray(
        np.asarray(inp["out_w"], np.float32).T.astype(ml_dtypes.bfloat16))
    if FOLD_D_HOST:
        w["w_out_D"] = np.ascontiguousarray(
            (np.asarray(inp["D_skip"], np.float32)[:, None]
             * np.asarray(inp["out_w"], np.float32).T
             ).astype(ml_dtypes.bfloat16))
    w["A_neg"] = f(-np.exp(np.asarray(inp["A_log"], np.float64)))  # (DIN, N)
    w["b_dt"] = f(inp["dt_b"][None, :])                # (1, DIN)
    w["b_out"] = f(inp["out_b"][None, :])              # (1, DM)
    w["d_col"] = f(np.asarray(inp["D_skip"])[:, None])  # (DIN, 1)
    w["lnw"] = f(np.concatenate(
        [inp["dtln_w"], inp["Bln_w"], inp["Cln_w"]])[:, None])  # (64, 1)
    m_ms = np.zeros((DT + 2 * N, 3), np.float32)
    m_ms[:DT, 0] = 1.0 / DT
    m_ms[DT:DT + N, 1] = 1.0 / N
    m_ms[DT + N:, 2] = 1.0 / N
    w["m_ms"] = m_ms
    e_bc = np.zeros((3, DT + 2 * N), np.float32)
    e_bc[0, :DT] = 1.0
    e_bc[1, DT:DT + N] = 1.0
    e_bc[2, DT + N:] = 1.0
    w["e_bc"] = e_bc
    w["ones_row"] = np.ones((1, TC), np.float32)
    return w


def host_x_shard(x, core):
    """x (B, L, DM) -> per-core transposed bf16 shard (BPC, DM, L)."""
    xs = np.asarray(x, np.float32)[core * BPC:(core + 1) * BPC]
    return np.ascontiguousarray(
        xs.transpose(0, 2, 1).astype(ml_dtypes.bfloat16))


# ---------------------------------------------------------------- IO decl
def declare_ios(nc):
    def d(name, shape=None, dt=F32):
        return nc.dram_tensor(name, list(shape), dt,
                              kind="ExternalInput").ap()
    ins = {
        "xT": d("xT", dt=BF16, shape= (BPC, DM, L)),
        "wk0": d("wk0", dt=BF16, shape= (DM, DIN)),
        "wk1": d("wk1", dt=BF16, shape= (DM, DIN)),
        "wk2": d("wk2", dt=BF16, shape= (DM, DIN)),
        "b_eff": d("b_eff", dt=MMDT, shape= (1, DIN)),
        "bfix": d("bfix", dt=MMDT, shape= (2, DIN)),
        "eye2": d("eye2", dt=MMDT, shape= (2, 2)),
        "w_x_T": d("w_x_T", dt=BF16, shape= (DIN, DT + 2 * N)),
        "w_dt_T": d("w_dt_T", dt=BF16, shape= (DT, DIN)),
        "w_out_T": d("w_out_T", dt=BF16, shape= (DIN, DM)),
        "A_neg": d("A_neg", (DIN, N)),
        "b_dt": d("b_dt", dt=MMDT, shape= (1, DIN)),
        "b_out": d("b_out", dt=MMDT, shape= (1, DM)),
        "d_col": d("d_col", (DIN, 1)),
        "lnw": d("lnw", (DT + 2 * N, 1)),
        "m_ms": d("m_ms", dt=MMDT, shape= (DT + 2 * N, 3)),
        "e_bc": d("e_bc", dt=MMDT, shape= (3, DT + 2 * N)),
        "ones_row": d("ones_row", dt=MMDT, shape= (1, TC)),
    }
    if FOLD_D_HOST:
        ins["w_out_D"] = d("w_out_D", (DIN, DM), dt=BF16)
    outs = {
        "y_out": nc.dram_tensor("y_out", [BPC, L, DM], F32,
                                kind="ExternalOutput").ap(),
    }
    return ins, outs


# ---------------------------------------------------------------- kernel body
def emit(tc_ctx, outs, ins):
    from contextlib import ExitStack
    tc = tc_ctx
    nc = tc.nc
    G = DT + 2 * N  # 64

    st = ExitStack()
    pool = lambda **kw: st.enter_context(tc.tile_pool(**kw))
    cpool = pool(name="consts", bufs=1)
    xpool = pool(name="xck", bufs=1)
    hpool = pool(name="h", bufs=2)
    trpool = pool(name="transient", bufs=2)
    spool = pool(name="smalls", bufs=1)
    dpool = pool(name="dlt", bufs=1)
    dApool = pool(name="dA", bufs=2)
    hspool = pool(name="hs", bufs=2)
    bxpool = pool(name="bx", bufs=2)
    bcpool = pool(name="bcb", bufs=1)
    ypool = pool(name="y", bufs=1)
    opool = pool(name="osb", bufs=1)
    pp_h = pool(name="ph", bufs=2, space="PSUM")
    pp_misc = pool(name="pmisc", bufs=2, space="PSUM")
    pp_d = pool(name="pd", bufs=2, space="PSUM")
    pp_o = pool(name="po", bufs=2, space="PSUM")

    dma = nc.sync.dma_start

    # ---- persistent constants -------------------------------------------
    def const_tile(name, shape=None, src=None, dt=F32):
        t = cpool.tile(list(shape), dt, tag=name)
        if src.dtype != dt and mybir.dt.size(src.dtype) == mybir.dt.size(dt):
            src = src.bitcast(dt)
        dma(t[:], src)
        return t

    wk_sb = [[const_tile(f"wk{k}_{kt}", (128, DIN),
                         ins[f"wk{k}"][kt * 128:(kt + 1) * 128, :], dt=BF16)
              for kt in range(NKT)] for k in range(K)]
    xproj_wT = [const_tile(f"xp_wT{k}", (128, G),
                           ins["w_x_T"][k * 128:(k + 1) * 128, :], dt=BF16)
                for k in range(NDT)]
    dt_wT = const_tile("dt_wT", (DT, DIN), ins["w_dt_T"][:, :], dt=BF16)
    out_wT = [const_tile(f"out_wT{k}", (128, DM),
                         ins["w_out_T"][k * 128:(k + 1) * 128, :], dt=BF16)
              for k in range(NDT)]
    if FOLD_D_HOST:
        out_wD = [const_tile(f"out_wD{k}", (128, DM),
                             ins["w_out_D"][k * 128:(k + 1) * 128, :],
                             dt=BF16)
                  for k in range(NDT)]
    A_sb = [const_tile(f"A{k}", (128, N),
                       ins["A_neg"][k * 128:(k + 1) * 128, :])
            for k in range(NDT)]
    d_col = [const_tile(f"D{k}", (128, 1),
                        ins["d_col"][k * 128:(k + 1) * 128, :])
             for k in range(NDT)]
    b_eff = const_tile("b_eff", dt=MMDT, shape=(1, DIN), src=ins["b_eff"][:, :])
    bfix = const_tile("bfix", dt=MMDT, shape=(2, DIN), src=ins["bfix"][:, :])
    eye2 = const_tile("eye2", dt=MMDT, shape=(2, 2), src=ins["eye2"][:, :])
    b_dt = const_tile("b_dt", dt=MMDT, shape=(1, DIN), src=ins["b_dt"][:, :])
    b_out = const_tile("b_out", dt=MMDT, shape=(1, DM), src=ins["b_out"][:, :])
    lnw = const_tile("lnw", (G, 1), ins["lnw"][:, :])
    m_ms = const_tile("m_ms", (G, 3), ins["m_ms"][:, :], dt=MMDT)
    e_bc = const_tile("e_bc", (3, G), ins["e_bc"][:, :], dt=MMDT)
    ones = const_tile("ones", (1, TC), ins["ones_row"][:, :], dt=MMDT)

    eps = cpool.tile([128, 1], F32, tag="eps")
    nc.vector.memset(eps[:], 1e-5)

    # persistent cross-chunk state
    state = cpool.tile([128, NDT * N], F32, tag="state")      # scan carries

    # DRAM bounce buffer for the B/C broadcast
    bc_dram = nc.dram_tensor("bc_scratch", [NCH, NH, 2 * NPH, TC], BF16).ap()

    for ch in range(NCH):
        bb, cb = divmod(ch, CPB)

        # ---- load x chunk with 2-col halo (already transposed on host) ---
        xck = []
        for kt in range(NKT):
            t = xpool.tile([128, TC + 2], BF16, tag=f"x{kt}")
            if cb == 0:
                nc.vector.memset(t[:, 0:2], 0.0)
                dma(t[:, 2:TC + 2], ins["xT"][bb, kt * 128:(kt + 1) * 128,
                                              0:TC])
            else:
                dma(t[:], ins["xT"][bb, kt * 128:(kt + 1) * 128,
                                    cb * TC - 2:cb * TC + TC])
            xck.append(t)

        # ---- stage A/B: fused in_proj+conv (PE) -> silu via tanh ---------
        h_list = []
        for dt in range(NDT):
            ph = pp_h.tile([128, TC], F32, tag="ph")
            ds = slice(dt * 128, (dt + 1) * 128)
            first = True
            for k in range(K):
                for kt in range(NKT):
                    nc.tensor.matmul(
                        ph[:], _r(wk_sb[k][kt][:, ds]),
                        _r(xck[kt][:, k:k + TC]), start=first, stop=False)
                    first = False
            if cb == 0:
                nc.tensor.matmul(ph[:, 0:2], _r(bfix[:, ds]),
                                 _r(eye2[:]), start=False, stop=False)
            nc.tensor.matmul(ph[:], _r(b_eff[0:1, ds]),
                             _r(ones[0:1, 0:TC]), start=False, stop=True)
            # h = silu(2*ph) directly on ACT (scale undoes the 0.5 prescale)
            h_t = hpool.tile([128, TC], BF16, tag=f"h{dt}")
            nc.scalar.activation(h_t[:], ph[:], AF.Silu, scale=2.0)
            h_list.append(h_t)

        # ---- stage C: x_proj + grouped rmsnorm ---------------------------
        pdbc = pp_misc.tile([G, TC], F32, tag="pmisc")
        for kt in range(NDT):
            nc.tensor.matmul(pdbc[:], _r(xproj_wT[kt][:]), _r(h_list[kt][:]),
                             start=(kt == 0), stop=(kt == NDT - 1))
        dbc_sb = spool.tile([G, TC], F32, tag="dbc")
        nc.scalar.copy(dbc_sb[:], pdbc[:])
        sq = spool.tile([G, TC], MMDT, tag="sq")
        nc.scalar.activation(sq[:], pdbc[:], AF.Square)
        pms = pp_misc.tile([3, TC], F32, tag="pmisc")
        nc.tensor.matmul(pms[:], _r(m_ms[:]), _r(sq[:]), start=True, stop=True)
        lnm = spool.tile([3, TC], F32, tag="lnm")
        nc.scalar.activation(lnm[:], pms[:], AF.Ln, bias=eps[0:3, :])
        rin = spool.tile([3, TC], MMDT, tag="rin")
        nc.scalar.activation(rin[:], lnm[:], AF.Exp, scale=-0.5)
        pr = pp_misc.tile([G, TC], F32, tag="pmisc")
        nc.tensor.matmul(pr[:], _r(e_bc[:]), _r(rin[:]), start=True, stop=True)
        delta_n = spool.tile([DT, TC], BF16, tag="dn")
        nc.vector.scalar_tensor_tensor(
            delta_n[:], dbc_sb[0:DT, :], lnw[0:DT, :], pr[0:DT, :],
            op0=ALU.mult, op1=ALU.mult)
        bc_n = spool.tile([2 * N, TC], BF16, tag="bcn")
        nc.vector.scalar_tensor_tensor(
            bc_n[:], dbc_sb[DT:G, :], lnw[DT:G, :], pr[DT:G, :],
            op0=ALU.mult, op1=ALU.mult)

        # bounce B/C rows through DRAM to broadcast across 128 partitions
        # quarter-major: [B rows of quarter | C rows of quarter]
        for q in range(NH):
            dma(bc_dram[ch, q, 0:NPH], bc_n[q * NPH:(q + 1) * NPH, :])
            dma(bc_dram[ch, q, NPH:2 * NPH],
                bc_n[N + q * NPH:N + (q + 1) * NPH, :])

        # per-quarter broadcast tiles (B rows then C rows, NPH each)
        bcq_list = []
        for q in range(NH):
            bcq = bcpool.tile([128, 2 * NPH * TC], BF16, tag=f"bcq{q}")
            nc.sync.dma_start(
                bcq[:].rearrange("p (j t) -> p j t", j=2 * NPH),
                bc_dram[ch, q].unsqueeze(0).broadcast_to(
                    (128, 2 * NPH, TC)))
            bcq_list.append(bcq)

        # ---- stage D/E: dt_proj, softplus, deltaA, scan, y ---------------
        # phase 1: all dt_proj matmuls + softplus-exp (exp_and_others set)
        esp_list = []
        for dt in range(NDT):
            pd = pp_d.tile([128, TC], F32, tag="pd")
            nc.tensor.matmul(pd[:],
                             _r(dt_wT[:, dt * 128:(dt + 1) * 128]),
                             _r(delta_n[:]), start=True, stop=False)
            nc.tensor.matmul(pd[:], _r(b_dt[0:1, dt * 128:(dt + 1) * 128]),
                             _r(ones[0:1, 0:TC]), start=False, stop=True)
            esp = dpool.tile([128, TC], BF16, tag=f"esp{dt}")
            nc.scalar.activation(esp[:], pd[:], AF.Exp)
            esp_list.append(esp)
        # phase 2: all softplus-ln (natural_log set) + u products
        dl_list = []
        ul_list = []
        for dt in range(NDT):
            delta_t = dpool.tile([128, TC], BF16, tag=f"dlt{dt}")
            nc.scalar.activation(delta_t[:], esp_list[dt][:], AF.Ln, bias=1.0)
            u_t = dpool.tile([128, TC], BF16, tag=f"u{dt}")
            nc.vector.tensor_mul(u_t[:], delta_t[:], h_list[dt][:])
            dl_list.append(delta_t)
            ul_list.append(u_t)
        # phase 3: deltaA exps + scans + readout
        y_list = []
        for dt in range(NDT):
            delta_t = dl_list[dt]
            u_t = ul_list[dt]

            y_t = ypool.tile([128, TC], BF16, tag=f"y{dt}")
            for qf in range(NH):
                bcq = bcq_list[qf]
                dA = dApool.tile([128, NPH * TC], BF16, tag="dA")
                hs = hspool.tile([128, NPH * TC], BF16, tag="hs")
                for n in range(NPH):
                    ng = qf * NPH + n
                    nc.scalar.activation(
                        dA[:, n * TC:(n + 1) * TC], delta_t[:], AF.Exp,
                        scale=A_sb[dt][:, ng:ng + 1])
                bx = bxpool.tile([128, NPH * TC], BF16, tag="bx")
                bx3 = bx[:].rearrange("p (n t) -> p n t", n=NPH)
                u3 = u_t[:].unsqueeze(1).broadcast_to((128, NPH, TC))
                nc.vector.tensor_mul(
                    bx3, u3,
                    bcq[:, 0:NPH * TC].rearrange("p (n t) -> p n t", n=NPH))
                if cb == 0:
                    # zero decay at inner segment starts, then one fused
                    # scan across all NPH segments (init=0 everywhere)
                    dA3 = dA[:].rearrange("p (n t) -> p n t", n=NPH)
                    nc.vector.memset(dA3[:, 1:NPH, 0:1], 0.0)
                    nc.vector.tensor_tensor_scan(
                        hs[:, :], dA[:, :], bx[:, :], 0.0,
                        op0=ALU.mult, op1=ALU.add)
                else:
                    # fold per-segment init into bx col 0, zero decay at
                    # every segment start, then one fused scan
                    dA3 = dA[:].rearrange("p (n t) -> p n t", n=NPH)
                    bx3v = bx[:].rearrange("p (n t) -> p n t", n=NPH)
                    a0 = trpool.tile([128, NPH], F32, tag="a0")
                    nc.vector.tensor_copy(a0[:], dA3[:, :, 0])
                    nc.vector.memset(dA3[:, :, 0:1], 0.0)
                    nc.vector.tensor_mul(
                        a0[:], a0[:],
                        state[:, dt * N + qf * NPH:dt * N + (qf + 1) * NPH])
                    nc.vector.tensor_add(bx3v[:, :, 0], bx3v[:, :, 0], a0[:])
                    nc.vector.tensor_tensor_scan(
                        hs[:, :], dA[:, :], bx[:, :], 0.0,
                        op0=ALU.mult, op1=ALU.add)
                if cb < CPB - 1:
                    hs3s = hs[:].rearrange("p (n t) -> p n t", n=NPH)
                    nc.vector.tensor_copy(
                        state[:, dt * N + qf * NPH:dt * N + (qf + 1) * NPH],
                        hs3s[:, :, TC - 1])
                # y contribution: multiply by C and tree-reduce over n
                hs3 = hs[:].rearrange("p (n t) -> p n t", n=NPH)
                nc.vector.tensor_mul(
                    hs3, hs3,
                    bcq[:, NPH * TC:2 * NPH * TC].rearrange(
                        "p (n t) -> p n t", n=NPH))
                m = NPH
                while m > 2:
                    half = m // 2
                    nc.vector.tensor_add(
                        hs[:, 0:half * TC].rearrange(
                            "p (n t) -> p n t", n=half),
                        hs[:, 0:half * TC].rearrange(
                            "p (n t) -> p n t", n=half),
                        hs[:, half * TC:m * TC].rearrange(
                            "p (n t) -> p n t", n=half))
                    m = half
                if qf == 0:
                    nc.vector.tensor_add(y_t[:], hs[:, 0:TC], hs[:, TC:2 * TC])
                else:
                    nc.vector.tensor_add(hs[:, 0:TC], hs[:, 0:TC],
                                         hs[:, TC:2 * TC])
                    nc.vector.tensor_add(y_t[:], y_t[:], hs[:, 0:TC])
            if not FOLD_D_HOST:
                nc.vector.scalar_tensor_tensor(
                    y_t[:], h_list[dt][:], d_col[dt][:], y_t[:],
                    op0=ALU.mult, op1=ALU.add)
            y_list.append(y_t)

        # ---- stage F: out_proj ------------------------------------------
        for tt in range(TC // 128):
            po = pp_o.tile([128, DM], F32, tag="po")
            for dt in range(NDT):
                nc.tensor.matmul(
                    po[:], _r(y_list[dt][:, tt * 128:(tt + 1) * 128]),
                    _r(out_wT[dt][:]), start=(dt == 0), stop=False)
            if FOLD_D_HOST:
                for dt in range(NDT):
                    nc.tensor.matmul(
                        po[:], _r(h_list[dt][:, tt * 128:(tt + 1) * 128]),
                        _r(out_wD[dt][:]), start=False, stop=False)
            nc.tensor.matmul(po[:], _r(ones[0:1, 0:128]), _r(b_out[0:1, :]),
                             start=False, stop=True)
            o_sb = opool.tile([128, DM], F32, tag="osb")
            nc.scalar.copy(o_sb[:], po[:])
            dma(outs["y_out"][bb, cb * TC + tt * 128:cb * TC + (tt + 1) * 128,
                              :], o_sb[:])

    st.close()


# ---------------------------------------------------------------- runner
_CACHE = {}


def _build_program():
    if "nc" in _CACHE:
        return _CACHE["nc"]
    nc = bacc.Bacc("TRN2", target_bir_lowering=False, debug=False,
                   num_devices=NCORES)
    ins, outs = declare_ios(nc)
    with tile.TileContext(nc) as t:
        emit(t, outs, ins)
    nc.compile()
    _CACHE["nc"] = nc
    return nc


LAST_RESULT = None


def kernel(**inputs) -> np.ndarray:
    global LAST_RESULT
    import os
    from concourse.bass_utils import run_bass_kernel_spmd

    nc = _build_program()
    w = host_weights(inputs)
    in_maps = []
    for c in range(NCORES):
        m = dict(w)
        m["xT"] = host_x_shard(inputs["x"], c)
        in_maps.append(m)
    trace = bool(os.environ.get("MIM_TRACE"))
    res = run_bass_kernel_spmd(nc, in_maps, list(range(NCORES)),
                               trace=trace)
    LAST_RESULT = res
    out = np.concatenate([res.results[c]["y_out"] for c in range(NCORES)],
                         axis=0)
    return out.astype(np.float32)



# revision 34
# speedup vs baseline: 1.1953x; 1.1953x over previous
"""Trainium2 Bass kernel for nn_MiM_v2 (Mamba-style selective scan).

Sharding: pure data-parallel over batch B=16 across 8 NeuronCores
(2 batches per core, weights replicated, no collectives).

Per-core pipeline over token chunks (TC tokens at a time):
  fused in_proj+causal-conv: 3 shifted bf16 matmuls with conv-prescaled
    weights + folded bias, x loaded with a 2-token halo (PE only)
  -> silu via tanh: h = ph*(1+tanh(ph)), ph prescaled by 0.5 (ACT+DVE STT;
     tanh shares the exp_and_others table set - no table thrash)
  -> x_proj (PE) -> grouped RMSNorm (PE reduce + ACT ln/exp + DVE)
  -> dt_proj (PE); softplus split into phases (all Exp, then all Ln) so
     the ACT table set flips only twice per chunk
  -> deltaA = exp(A*delta) (ACT, per-partition scale, bf16)
  -> BX = delta*h*B (DVE, broadcast AP vs DRAM-bounced B/C broadcast)
  -> linear recurrence via DVE tensor_tensor_scan, double-buffered
     dA/hs/bx pools for cross-iteration overlap; cross-chunk carries via
     per-n init columns copied from the previous chunk's scan tails
  -> y = sum_n hs*C (DVE mult + gpsimd/DVE tree reduce)  [+ D_skip*h]
  -> out_proj (bf16 PE) -> DMA out
"""

import sys

if "/opt/trn_rl_repo" not in sys.path:
    sys.path.insert(0, "/opt/trn_rl_repo")

import numpy as np
import ml_dtypes

import concourse.bass as bass
import concourse.mybir as mybir
import concourse.tile as tile
from concourse import bacc

# ---------------------------------------------------------------- constants
B, L, DM = 16, 1024, 512
DIN, DT, N, K = 2 * DM, 32, 16, 3
NCORES = 8
BPC = B // NCORES          # batches per core
T = BPC * L                # tokens per core
TC = 512                   # token chunk
NCH = T // TC              # chunks per core
CPB = L // TC              # chunks per batch
NDT = DIN // 128           # d-inner tiles
NKT = DM // 128            # k tiles for in_proj
NH = 2                     # n-groups for scan phase
GP_SCANS = 0               # gpsimd cannot run tensor_tensor_scan (codegen)
NPH = N // NH              # n per half (8)

F32 = mybir.dt.float32
F32R = mybir.dt.float32r
BF16 = mybir.dt.bfloat16
AF = mybir.ActivationFunctionType
ALU = mybir.AluOpType

FOLD_D_HOST = True         # fold D_skip into an extra out_proj matmul


USE_F32R = True
MMDT = None  # set below


MMDT = F32R if USE_F32R else F32


def _r(ap):
    """matmul operands already carry the right dtype; no bitcast needed"""
    return ap


# ---------------------------------------------------------------- host prep
def host_weights(inp):
    """Precompute transposed/reorganized weights (numpy, shared by all cores)."""
    f = lambda x: np.ascontiguousarray(np.asarray(x, np.float32))
    w = {}
    # conv folded into in_proj: h_conv[t,d] = sum_k cw[d,k]*(x[t+k-2]@in_w.T[:,d] + in_b[d]) + conv_b[d]
    # scaled by 0.5 so that silu(x)=x*sigmoid(x)=ph*(1+tanh(ph)) with ph=0.5*x
    cw = np.asarray(inp["conv_w"], np.float32)[:, 0, :]   # (DIN, 3)
    in_wT = np.asarray(inp["in_w"], np.float32).T          # (DM, DIN)
    for k in range(K):
        w[f"wk{k}"] = np.ascontiguousarray(
            (0.5 * in_wT * cw[None, :, k]).astype(ml_dtypes.bfloat16))
    in_b = np.asarray(inp["in_b"], np.float32)
    conv_b = np.asarray(inp["conv_b"], np.float32)
    w["b_eff"] = f(0.5 * (in_b * cw.sum(1) + conv_b))[None, :]       # (1, DIN)
    w["bfix"] = f(np.stack([-0.5 * in_b * (cw[:, 0] + cw[:, 1]),
                            -0.5 * in_b * cw[:, 0]]))               # (2, DIN)
    w["eye2"] = f(np.eye(2, dtype=np.float32))
    w["w_x_T"] = np.ascontiguousarray(
        np.asarray(inp["xproj_w"], np.float32).T.astype(ml_dtypes.bfloat16))
    w["w_dt_T"] = np.ascontiguousarray(
        np.asarray(inp["dt_w"], np.float32).T.astype(ml_dtypes.bfloat16))
    w["w_out_T"] = np.ascontiguousarray(
        np.asarray(inp["out_w"], np.float32).T.astype(ml_dtypes.bfloat16))
    if FOLD_D_HOST:
        w["w_out_D"] = np.ascontiguousarray(
            (np.asarray(inp["D_skip"], np.float32)[:, None]
             * np.asarray(inp["out_w"], np.float32).T
             ).astype(ml_dtypes.bfloat16))
    w["A_neg"] = f(-np.exp(np.asarray(inp["A_log"], np.float64)))  # (DIN, N)
    w["b_dt"] = f(inp["dt_b"][None, :])                # (1, DIN)
    w["b_out"] = f(inp["out_b"][None, :])              # (1, DM)
    w["d_col"] = f(np.asarray(inp["D_skip"])[:, None])  # (DIN, 1)
    w["lnw"] = f(np.concatenate(
        [inp["dtln_w"], inp["Bln_w"], inp["Cln_w"]])[:, None])  # (64, 1)
    m_ms = np.zeros((DT + 2 * N, 3), np.float32)
    m_ms[:DT, 0] = 1.0 / DT
    m_ms[DT:DT + N, 1] = 1.0 / N
    m_ms[DT + N:, 2] = 1.0 / N
    w["m_ms"] = m_ms
    e_bc = np.zeros((3, DT + 2 * N), np.float32)
    e_bc[0, :DT] = 1.0
    e_bc[1, DT:DT + N] = 1.0
    e_bc[2, DT + N:] = 1.0
    w["e_bc"] = e_bc
    w["ones_row"] = np.ones((1, TC), np.float32)
    return w


def host_x_shard(x, core):
    """x (B, L, DM) -> per-core transposed bf16 shard (BPC, DM, L)."""
    xs = np.asarray(x, np.float32)[core * BPC:(core + 1) * BPC]
    return np.ascontiguousarray(
        xs.transpose(0, 2, 1).astype(ml_dtypes.bfloat16))


# ---------------------------------------------------------------- IO decl
def declare_ios(nc):
    def d(name, shape=None, dt=F32):
        return nc.dram_tensor(name, list(shape), dt,
                              kind="ExternalInput").ap()
    ins = {
        "xT": d("xT", dt=BF16, shape= (BPC, DM, L)),
        "wk0": d("wk0", dt=BF16, shape= (DM, DIN)),
        "wk1": d("wk1", dt=BF16, shape= (DM, DIN)),
        "wk2": d("wk2", dt=BF16, shape= (DM, DIN)),
        "b_eff": d("b_eff", dt=MMDT, shape= (1, DIN)),
        "bfix": d("bfix", dt=MMDT, shape= (2, DIN)),
        "eye2": d("eye2", dt=MMDT, shape= (2, 2)),
        "w_x_T": d("w_x_T", dt=BF16, shape= (DIN, DT + 2 * N)),
        "w_dt_T": d("w_dt_T", dt=BF16, shape= (DT, DIN)),
        "w_out_T": d("w_out_T", dt=BF16, shape= (DIN, DM)),
        "A_neg": d("A_neg", (DIN, N)),
        "b_dt": d("b_dt", dt=MMDT, shape= (1, DIN)),
        "b_out": d("b_out", dt=MMDT, shape= (1, DM)),
        "d_col": d("d_col", (DIN, 1)),
        "lnw": d("lnw", (DT + 2 * N, 1)),
        "m_ms": d("m_ms", dt=MMDT, shape= (DT + 2 * N, 3)),
        "e_bc": d("e_bc", dt=MMDT, shape= (3, DT + 2 * N)),
        "ones_row": d("ones_row", dt=MMDT, shape= (1, TC)),
    }
    if FOLD_D_HOST:
        ins["w_out_D"] = d("w_out_D", (DIN, DM), dt=BF16)
    outs = {
        "y_out": nc.dram_tensor("y_out", [BPC, L, DM], F32,
                                kind="ExternalOutput").ap(),
    }
    return ins, outs


# ---------------------------------------------------------------- kernel body
def emit(tc_ctx, outs, ins):
    from contextlib import ExitStack
    tc = tc_ctx
    nc = tc.nc
    G = DT + 2 * N  # 64

    st = ExitStack()
    pool = lambda **kw: st.enter_context(tc.tile_pool(**kw))
    cpool = pool(name="consts", bufs=1)
    xpool = pool(name="xck", bufs=1)
    hpool = pool(name="h", bufs=2)
    trpool = pool(name="transient", bufs=2)
    spool = pool(name="smalls", bufs=1)
    dpool = pool(name="dlt", bufs=1)
    dApool = pool(name="dA", bufs=2)
    hspool = pool(name="hs", bufs=2)
    bxpool = pool(name="bx", bufs=2)
    bcpool = pool(name="bcb", bufs=1)
    ypool = pool(name="y", bufs=1)
    opool = pool(name="osb", bufs=1)
    pp_h = pool(name="ph", bufs=1, space="PSUM")
    pp_misc = pool(name="pmisc", bufs=2, space="PSUM")
    pp_d = pool(name="pd", bufs=2, space="PSUM")
    pp_o = pool(name="po", bufs=2, space="PSUM")

    dma = nc.sync.dma_start

    # ---- persistent constants -------------------------------------------
    def const_tile(name, shape=None, src=None, dt=F32):
        t = cpool.tile(list(shape), dt, tag=name)
        if src.dtype != dt and mybir.dt.size(src.dtype) == mybir.dt.size(dt):
            src = src.bitcast(dt)
        dma(t[:], src)
        return t

    wk_sb = [[const_tile(f"wk{k}_{kt}", (128, DIN),
                         ins[f"wk{k}"][kt * 128:(kt + 1) * 128, :], dt=BF16)
              for kt in range(NKT)] for k in range(K)]
    xproj_wT = [const_tile(f"xp_wT{k}", (128, G),
                           ins["w_x_T"][k * 128:(k + 1) * 128, :], dt=BF16)
                for k in range(NDT)]
    dt_wT = const_tile("dt_wT", (DT, DIN), ins["w_dt_T"][:, :], dt=BF16)
    out_wT = [const_tile(f"out_wT{k}", (128, DM),
                         ins["w_out_T"][k * 128:(k + 1) * 128, :], dt=BF16)
              for k in range(NDT)]
    if FOLD_D_HOST:
        out_wD = [const_tile(f"out_wD{k}", (128, DM),
                             ins["w_out_D"][k * 128:(k + 1) * 128, :],
                             dt=BF16)
                  for k in range(NDT)]
    A_sb = [const_tile(f"A{k}", (128, N),
                       ins["A_neg"][k * 128:(k + 1) * 128, :])
            for k in range(NDT)]
    d_col = [const_tile(f"D{k}", (128, 1),
                        ins["d_col"][k * 128:(k + 1) * 128, :])
             for k in range(NDT)]
    b_eff = const_tile("b_eff", dt=MMDT, shape=(1, DIN), src=ins["b_eff"][:, :])
    bfix = const_tile("bfix", dt=MMDT, shape=(2, DIN), src=ins["bfix"][:, :])
    eye2 = const_tile("eye2", dt=MMDT, shape=(2, 2), src=ins["eye2"][:, :])
    b_dt = const_tile("b_dt", dt=MMDT, shape=(1, DIN), src=ins["b_dt"][:, :])
    b_out = const_tile("b_out", dt=MMDT, shape=(1, DM), src=ins["b_out"][:, :])
    lnw = const_tile("lnw", (G, 1), ins["lnw"][:, :])
    m_ms = const_tile("m_ms", (G, 3), ins["m_ms"][:, :], dt=MMDT)
    e_bc = const_tile("e_bc", (3, G), ins["e_bc"][:, :], dt=MMDT)
    ones = const_tile("ones", (1, TC), ins["ones_row"][:, :], dt=MMDT)

    eps = cpool.tile([128, 1], F32, tag="eps")
    nc.vector.memset(eps[:], 1e-5)

    # persistent cross-chunk state
    state = cpool.tile([128, NDT * N], F32, tag="state")      # scan carries

    # DRAM bounce buffer for the B/C broadcast
    bc_dram = nc.dram_tensor("bc_scratch", [NCH, NH, 2 * NPH, TC], BF16).ap()

    for ch in range(NCH):
        bb, cb = divmod(ch, CPB)

        # ---- load x chunk with 2-col halo (already transposed on host) ---
        xck = []
        for kt in range(NKT):
            t = xpool.tile([128, TC + 2], BF16, tag=f"x{kt}")
            if cb == 0:
                nc.vector.memset(t[:, 0:2], 0.0)
                dma(t[:, 2:TC + 2], ins["xT"][bb, kt * 128:(kt + 1) * 128,
                                              0:TC])
            else:
                dma(t[:], ins["xT"][bb, kt * 128:(kt + 1) * 128,
                                    cb * TC - 2:cb * TC + TC])
            xck.append(t)

        # ---- stage A/B: fused in_proj+conv (PE) -> silu via tanh ---------
        h_list = []
        for dt in range(NDT):
            ph = pp_h.tile([128, TC], F32, tag="ph")
            ds = slice(dt * 128, (dt + 1) * 128)
            first = True
            for k in range(K):
                for kt in range(NKT):
                    nc.tensor.matmul(
                        ph[:], _r(wk_sb[k][kt][:, ds]),
                        _r(xck[kt][:, k:k + TC]), start=first, stop=False)
                    first = False
            if cb == 0:
                nc.tensor.matmul(ph[:, 0:2], _r(bfix[:, ds]),
                                 _r(eye2[:]), start=False, stop=False)
            nc.tensor.matmul(ph[:], _r(b_eff[0:1, ds]),
                             _r(ones[0:1, 0:TC]), start=False, stop=True)
            # h = silu(2*ph) directly on ACT (scale undoes the 0.5 prescale)
            h_t = hpool.tile([128, TC], BF16, tag=f"h{dt}")
            nc.scalar.activation(h_t[:], ph[:], AF.Silu, scale=2.0)
            h_list.append(h_t)

        # ---- stage C: x_proj + grouped rmsnorm ---------------------------
        pdbc = pp_misc.tile([G, TC], F32, tag="pmisc")
        for kt in range(NDT):
            nc.tensor.matmul(pdbc[:], _r(xproj_wT[kt][:]), _r(h_list[kt][:]),
                             start=(kt == 0), stop=(kt == NDT - 1))
        dbc_sb = spool.tile([G, TC], F32, tag="dbc")
        nc.scalar.copy(dbc_sb[:], pdbc[:])
        sq = spool.tile([G, TC], MMDT, tag="sq")
        nc.scalar.activation(sq[:], pdbc[:], AF.Square)
        pms = pp_misc.tile([3, TC], F32, tag="pmisc")
        nc.tensor.matmul(pms[:], _r(m_ms[:]), _r(sq[:]), start=True, stop=True)
        lnm = spool.tile([3, TC], F32, tag="lnm")
        nc.scalar.activation(lnm[:], pms[:], AF.Ln, bias=eps[0:3, :])
        rin = spool.tile([3, TC], MMDT, tag="rin")
        nc.scalar.activation(rin[:], lnm[:], AF.Exp, scale=-0.5)
        pr = pp_misc.tile([G, TC], F32, tag="pmisc")
        nc.tensor.matmul(pr[:], _r(e_bc[:]), _r(rin[:]), start=True, stop=True)
        delta_n = spool.tile([DT, TC], BF16, tag="dn")
        nc.vector.scalar_tensor_tensor(
            delta_n[:], dbc_sb[0:DT, :], lnw[0:DT, :], pr[0:DT, :],
            op0=ALU.mult, op1=ALU.mult)
        bc_n = spool.tile([2 * N, TC], BF16, tag="bcn")
        nc.vector.scalar_tensor_tensor(
            bc_n[:], dbc_sb[DT:G, :], lnw[DT:G, :], pr[DT:G, :],
            op0=ALU.mult, op1=ALU.mult)

        # bounce B/C rows through DRAM to broadcast across 128 partitions
        # quarter-major: [B rows of quarter | C rows of quarter]
        for q in range(NH):
            dma(bc_dram[ch, q, 0:NPH], bc_n[q * NPH:(q + 1) * NPH, :])
            dma(bc_dram[ch, q, NPH:2 * NPH],
                bc_n[N + q * NPH:N + (q + 1) * NPH, :])

        # per-quarter broadcast tiles (B rows then C rows, NPH each)
        bcq_list = []
        for q in range(NH):
            bcq = bcpool.tile([128, 2 * NPH * TC], BF16, tag=f"bcq{q}")
            nc.sync.dma_start(
                bcq[:].rearrange("p (j t) -> p j t", j=2 * NPH),
                bc_dram[ch, q].unsqueeze(0).broadcast_to(
                    (128, 2 * NPH, TC)))
            bcq_list.append(bcq)

        # ---- stage D/E: dt_proj, softplus, deltaA, scan, y ---------------
        # phase 1: all dt_proj matmuls + softplus-exp (exp_and_others set)
        esp_list = []
        for dt in range(NDT):
            pd = pp_d.tile([128, TC], F32, tag="pd")
            nc.tensor.matmul(pd[:],
                             _r(dt_wT[:, dt * 128:(dt + 1) * 128]),
                             _r(delta_n[:]), start=True, stop=False)
            nc.tensor.matmul(pd[:], _r(b_dt[0:1, dt * 128:(dt + 1) * 128]),
                             _r(ones[0:1, 0:TC]), start=False, stop=True)
            esp = dpool.tile([128, TC], BF16, tag=f"esp{dt}")
            nc.scalar.activation(esp[:], pd[:], AF.Exp)
            esp_list.append(esp)
        # phase 2: all softplus-ln (natural_log set) + u products
        dl_list = []
        ul_list = []
        for dt in range(NDT):
            delta_t = dpool.tile([128, TC], BF16, tag=f"dlt{dt}")
            nc.scalar.activation(delta_t[:], esp_list[dt][:], AF.Ln, bias=1.0)
            u_t = dpool.tile([128, TC], BF16, tag=f"u{dt}")
            nc.vector.tensor_mul(u_t[:], delta_t[:], h_list[dt][:])
            dl_list.append(delta_t)
            ul_list.append(u_t)
        # phase 3: deltaA exps + scans + readout
        y_list = []
        for dt in range(NDT):
            delta_t = dl_list[dt]
            u_t = ul_list[dt]

            y_t = ypool.tile([128, TC], BF16, tag=f"y{dt}")
            for qf in range(NH):
                bcq = bcq_list[qf]
                dA = dApool.tile([128, NPH * TC], BF16, tag="dA")
                hs = hspool.tile([128, NPH * TC], BF16, tag="hs")
                for n in range(NPH):
                    ng = qf * NPH + n
                    nc.scalar.activation(
                        dA[:, n * TC:(n + 1) * TC], delta_t[:], AF.Exp,
                        scale=A_sb[dt][:, ng:ng + 1])
                bx = bxpool.tile([128, NPH * TC], BF16, tag="bx")
                bx3 = bx[:].rearrange("p (n t) -> p n t", n=NPH)
                u3 = u_t[:].unsqueeze(1).broadcast_to((128, NPH, TC))
                nc.vector.tensor_mul(
                    bx3, u3,
                    bcq[:, 0:NPH * TC].rearrange("p (n t) -> p n t", n=NPH))
                if cb == 0:
                    # zero decay at inner segment starts, then one fused
                    # scan across all NPH segments (init=0 everywhere)
                    dA3 = dA[:].rearrange("p (n t) -> p n t", n=NPH)
                    nc.vector.memset(dA3[:, 1:NPH, 0:1], 0.0)
                    nc.vector.tensor_tensor_scan(
                        hs[:, :], dA[:, :], bx[:, :], 0.0,
                        op0=ALU.mult, op1=ALU.add)
                else:
                    # fold per-segment init into bx col 0, zero decay at
                    # every segment start, then one fused scan
                    dA3 = dA[:].rearrange("p (n t) -> p n t", n=NPH)
                    bx3v = bx[:].rearrange("p (n t) -> p n t", n=NPH)
                    a0 = trpool.tile([128, NPH], F32, tag="a0")
                    nc.vector.tensor_copy(a0[:], dA3[:, :, 0])
                    nc.vector.memset(dA3[:, :, 0:1], 0.0)
                    nc.vector.tensor_mul(
                        a0[:], a0[:],
                        state[:, dt * N + qf * NPH:dt * N + (qf + 1) * NPH])
                    nc.vector.tensor_add(bx3v[:, :, 0], bx3v[:, :, 0], a0[:])
                    nc.vector.tensor_tensor_scan(
                        hs[:, :], dA[:, :], bx[:, :], 0.0,
                        op0=ALU.mult, op1=ALU.add)
                if cb < CPB - 1:
                    hs3s = hs[:].rearrange("p (n t) -> p n t", n=NPH)
                    nc.vector.tensor_copy(
                        state[:, dt * N + qf * NPH:dt * N + (qf + 1) * NPH],
                        hs3s[:, :, TC - 1])
                # y contribution: multiply by C and tree-reduce over n
                hs3 = hs[:].rearrange("p (n t) -> p n t", n=NPH)
                nc.vector.tensor_mul(
                    hs3, hs3,
                    bcq[:, NPH * TC:2 * NPH * TC].rearrange(
                        "p (n t) -> p n t", n=NPH))
                m = NPH
                while m > 2:
                    half = m // 2
                    nc.vector.tensor_add(
                        hs[:, 0:half * TC].rearrange(
                            "p (n t) -> p n t", n=half),
                        hs[:, 0:half * TC].rearrange(
                            "p (n t) -> p n t", n=half),
                        hs[:, half * TC:m * TC].rearrange(
                            "p (n t) -> p n t", n=half))
                    m = half
                if qf == 0:
                    nc.vector.tensor_add(y_t[:], hs[:, 0:TC], hs[:, TC:2 * TC])
                else:
                    nc.vector.tensor_add(hs[:, 0:TC], hs[:, 0:TC],
                                         hs[:, TC:2 * TC])
                    nc.vector.tensor_add(y_t[:], y_t[:], hs[:, 0:TC])
            if not FOLD_D_HOST:
                nc.vector.scalar_tensor_tensor(
                    y_t[:], h_list[dt][:], d_col[dt][:], y_t[:],
                    op0=ALU.mult, op1=ALU.add)
            y_list.append(y_t)

        # ---- stage F: out_proj ------------------------------------------
        for tt in range(TC // 128):
            po = pp_o.tile([128, DM], F32, tag="po")
            for dt in range(NDT):
                nc.tensor.matmul(
                    po[:], _r(y_list[dt][:, tt * 128:(tt + 1) * 128]),
                    _r(out_wT[dt][:]), start=(dt == 0), stop=False)
            if FOLD_D_HOST:
                for dt in range(NDT):
                    nc.tensor.matmul(
                        po[:], _r(h_list[dt][:, tt * 128:(tt + 1) * 128]),
                        _r(out_wD[dt][:]), start=False, stop=False)
            nc.tensor.matmul(po[:], _r(ones[0:1, 0:128]), _r(b_out[0:1, :]),
                             start=False, stop=True)
            o_sb = opool.tile([128, DM], F32, tag="osb")
            nc.scalar.copy(o_sb[:], po[:])
            dma(outs["y_out"][bb, cb * TC + tt * 128:cb * TC + (tt + 1) * 128,
                              :], o_sb[:])

    st.close()


# ---------------------------------------------------------------- runner
_CACHE = {}


def _build_program():
    if "nc" in _CACHE:
        return _CACHE["nc"]
    nc = bacc.Bacc("TRN2", target_bir_lowering=False, debug=False,
                   num_devices=NCORES)
    ins, outs = declare_ios(nc)
    with tile.TileContext(nc) as t:
        emit(t, outs, ins)
    nc.compile()
    _CACHE["nc"] = nc
    return nc


LAST_RESULT = None


def kernel(**inputs) -> np.ndarray:
    global LAST_RESULT
    import os
    from concourse.bass_utils import run_bass_kernel_spmd

    nc = _build_program()
    w = host_weights(inputs)
    in_maps = []
    for c in range(NCORES):
        m = dict(w)
        m["xT"] = host_x_shard(inputs["x"], c)
        in_maps.append(m)
    trace = bool(os.environ.get("MIM_TRACE"))
    res = run_bass_kernel_spmd(nc, in_maps, list(range(NCORES)),
                               trace=trace)
    LAST_RESULT = res
    out = np.concatenate([res.results[c]["y_out"] for c in range(NCORES)],
                         axis=0)
    return out.astype(np.float32)

